# revision 1
# baseline (speedup 1.0000x reference)
"""Trainium2 kernel for nn_AttentionFusion (dense_transformer).

Math: the reference MHA has seq_len 1 for q and kv, so softmax over the
single kv position is identically 1.0 and the attention output equals the
value projection. The whole module therefore collapses (exactly, up to fp
rounding) to one affine map per input stream:

    out = relu(audio @ Waa.T + visual @ Wva.T + b)

with
    Wvo = Wo @ Wi[2E:]             bvo = Wo @ bi[2E:] + bo
    Wfv = Wf[:, :E] @ Wvo          Wfa = Wf[:, E:] @ Wvo
    Waa = Wfa @ Wa                 Wva = Wfv @ Wv
    b   = Wfa @ ba + Wfv @ bv + (Wf[:, :E] + Wf[:, E:]) @ bvo + bf

Weight composition is done on host in float64 (cheap: ~15 GFLOP), the big
GEMM (32768 x 4096 @ 4096 x 1024, 275 GFLOP) runs on 8 NeuronCores, batch
sharded (pure data parallel per the sharding hint).

Device layout per core:
    xt  [K=4096, BC=4096]  fp32  - per-core activations, feature-major
    wt  [K=4096, E=1024]   fp32  - composed weight, feature-major (replicated)
    bias[P=128,  E=1024]   fp32  - row-replicated bias
    out [BC=4096, E=1024]  fp32  - natural layout

Matmuls run as float32r (fp32 bits on the PE fast path: 1 cyc/row at
N>=256 vs 4 cyc/row for plain float32; ~1.3e-4 scale-relative absmax).
PSUM tile [128 batch, 512 outfeat]: stationary = xt subtile [128k, 128b]
(reused across the 2 outfeat halves to halve weight-load overhead),
moving = wt tile [128k, 512e]. Epilogue: DVE adds the row-replicated
bias PSUM->SBUF, ScalarE applies Relu, DMA out in natural layout.
"""

import os
import sys

import numpy as np

sys.path.insert(0, "/opt/trn_rl_repo")

import concourse.bacc as bacc
import concourse.mybir as mybir
import concourse.tile as tile
from concourse.bass_utils import run_bass_kernel_spmd

N_CORES = 8
B = 32768
BC = B // N_CORES  # 4096 batch rows per core
K = 4096           # 2048 audio + 2048 visual features
E = 1024
P = 128

KO = K // P        # 32 contraction tiles
NB = 512           # batch tile per iteration
NBT = BC // NB     # 8 batch tiles per core
B4 = NB // P       # 4 batch subtiles (PSUM partition dim)
M2 = E // NB       # 2 outfeat halves (PSUM free dim)

DT_NAME = os.environ.get("KMM_DTYPE", "f32r")

_NC_CACHE = {}
LAST_RESULTS = None  # stashed BassKernelResults for test.py introspection

# Note: walrus's --enable-ldw-opt=true was tested (dedupes the shared-lhsT
# LDWEIGHTS pairs, 2048 -> 1087) but measured SLOWER: the standalone-LW form
# loses the fused matmul's background weight-buffer pipelining (+12us PE).


def _build_nc(dt_name):
    mm_dt = {"f32": mybir.dt.float32, "f32r": mybir.dt.float32r}[dt_name]
    f32 = mybir.dt.float32

    nc = bacc.Bacc("TRN2", debug=False, target_bir_lowering=False)
    xt = nc.dram_tensor("xt", [K, BC], mm_dt, kind="ExternalInput").ap()
    wt = nc.dram_tensor("wt", [K, E], mm_dt, kind="ExternalInput").ap()
    bias = nc.dram_tensor("bias", [P, E], f32, kind="ExternalInput").ap()
    out = nc.dram_tensor("out", [BC, E], f32, kind="ExternalOutput").ap()

    with tile.TileContext(nc) as tc:
        with (
            tc.tile_pool(name="wpool", bufs=1) as wpool,
            tc.tile_pool(name="xpool", bufs=12) as xpool,
            tc.tile_pool(name="opool", bufs=8) as opool,
            tc.tile_pool(name="pspool", bufs=8, space="PSUM") as pspool,
        ):
            # The DMA path is one FIFO queue fanned over 16 engines (~300-346
            # GB/s measured; bigger transfers amortize per-DMA overhead):
            # arrival order == emission order. Order the preamble
            # just-in-time for batch tile 0's k-sweep: first 8 xch (PE can
            # start), weight chunks 0-7, then alternating (xch, weight)
            # pairs so neither stream starves the other. Emitting the whole
            # 16.8 MB weight load first stalls PE ~60 us.
            wt_sb = wpool.tile([P, KO, E], mm_dt)
            wt_r = wt.rearrange("(ko ki) e -> ki ko e", ki=P)
            xch0 = {}
            for k in range(8):
                xch = xpool.tile([P, NB], mm_dt, tag="xch")
                nc.sync.dma_start(xch, xt[k * P : (k + 1) * P, 0:NB])
                xch0[k] = xch
            for ko in range(8):
                nc.sync.dma_start(wt_sb[:, ko], wt_r[:, ko])

            bias_sb = wpool.tile([P, E], f32)
            for k in range(8, KO):
                xch = xpool.tile([P, NB], mm_dt, tag="xch")
                nc.sync.dma_start(xch, xt[k * P : (k + 1) * P, 0:NB])
                xch0[k] = xch
                if k % 4 == 0:
                    # 2 MB weight chunks (4 ko's): fewer, larger transfers
                    # raise effective DMA bandwidth in the bandwidth-bound
                    # first k-sweep.
                    nc.sync.dma_start(wt_sb[:, k : k + 4], wt_r[:, k : k + 4])
                if k == 11:
                    # Bias early enough for the first iteration's drains
                    # (~85us) but off the first weight chunks' critical path
                    # (row-replicated, [128, E]).
                    nc.sync.dma_start(bias_sb, bias)

            for n in range(NBT):
                psums = [
                    pspool.tile([P, NB], f32, tag="ps", name=f"ps_{n}_{j}")
                    for j in range(B4 * M2)
                ]
                for k in range(KO):
                    if n == 0:
                        xch = xch0[k]
                    else:
                        xch = xpool.tile([P, NB], mm_dt, tag="xch")
                        nc.sync.dma_start(
                            xch, xt[k * P : (k + 1) * P, n * NB : (n + 1) * NB]
                        )
                    for b in range(B4):
                        for m in range(M2):
                            nc.tensor.matmul(
                                psums[b * M2 + m],
                                lhsT=xch[:, b * P : (b + 1) * P],
                                rhs=wt_sb[:, k, m * NB : (m + 1) * NB],
                                start=(k == 0),
                                stop=(k == KO - 1),
                            )
                for b in range(B4):
                    for m in range(M2):
                        ps = psums[b * M2 + m]
                        osb = opool.tile([P, NB], f32, tag="osb")
                        nc.vector.tensor_add(
                            out=osb,
                            in0=ps,
                            in1=bias_sb[:, m * NB : (m + 1) * NB],
                        )
                        nc.scalar.activation(
                            osb, osb, mybir.ActivationFunctionType.Relu
                        )
                        nc.sync.dma_start(
                            out[
                                n * NB + b * P : n * NB + (b + 1) * P,
                                m * NB : (m + 1) * NB,
                            ],
                            osb,
                        )

    nc.compile()
    return nc


def _get_nc(dt_name):
    if dt_name not in _NC_CACHE:
        _NC_CACHE[dt_name] = _build_nc(dt_name)
    return _NC_CACHE[dt_name]


def _compose_weights(Wa, ba, Wv, bv, Wi, bi, Wo, bo, Wf, bf):
    f6 = lambda x: np.asarray(x, dtype=np.float64)
    Wvo = f6(Wo) @ f6(Wi[2 * E :])
    bvo = f6(Wo) @ f6(bi[2 * E :]) + f6(bo)
    Wf1, Wf2 = f6(Wf[:, :E]), f6(Wf[:, E:])
    Wfv = Wf1 @ Wvo  # applied to visual_e for audio_att
    Wfa = Wf2 @ Wvo  # applied to audio_e for visual_att
    Waa = Wfa @ f6(Wa)  # [E, 2048] applied to audio
    Wva = Wfv @ f6(Wv)  # [E, 2048] applied to visual
    b = Wfa @ f6(ba) + Wfv @ f6(bv) + (Wf1 + Wf2) @ bvo + f6(bf)
    wt = np.ascontiguousarray(
        np.concatenate([Waa, Wva], axis=1).T, dtype=np.float32
    )  # [K, E]
    return wt, b.astype(np.float32)


def kernel(audio, visual, Wa, ba, Wv, bv, Wi, bi, Wo, bo, Wf, bf):
    global LAST_RESULTS
    wt, bias = _compose_weights(Wa, ba, Wv, bv, Wi, bi, Wo, bo, Wf, bf)
    bias_bc = np.ascontiguousarray(np.broadcast_to(bias, (P, E)), np.float32)
    audio = np.asarray(audio, dtype=np.float32)
    visual = np.asarray(visual, dtype=np.float32)

    in_maps = []
    for c in range(N_CORES):
        rows = slice(c * BC, (c + 1) * BC)
        xt_c = np.empty((K, BC), np.float32)
        xt_c[: K // 2] = audio[rows].T
        xt_c[K // 2 :] = visual[rows].T
        in_maps.append({"xt": xt_c, "wt": wt, "bias": bias_bc})

    nc = _get_nc(DT_NAME)
    trace = os.environ.get("KMM_TRACE", "0") == "1"
    kwargs = {}
    if os.environ.get("KMM_TRACE_ALL", "0") == "1":
        kwargs["trace_cores"] = list(range(N_CORES))
    res = run_bass_kernel_spmd(
        nc, in_maps, core_ids=list(range(N_CORES)), trace=trace, **kwargs
    )
    LAST_RESULTS = res
    out = np.concatenate([r["out"] for r in res.results], axis=0)
    return np.ascontiguousarray(out, dtype=np.float32)



# revision 5
# speedup vs baseline: 1.2713x; 1.2713x over previous
"""Trainium2 kernel for nn_AttentionFusion (dense_transformer).

Math: the reference MHA has seq_len 1 for q and kv, so softmax over the
single kv position is identically 1.0 and the attention output equals the
value projection. The whole module therefore collapses (exactly, up to fp
rounding) to one affine map per input stream:

    out = relu(audio @ Waa.T + visual @ Wva.T + b)

with
    Wvo = Wo @ Wi[2E:]             bvo = Wo @ bi[2E:] + bo
    Wfv = Wf[:, :E] @ Wvo          Wfa = Wf[:, E:] @ Wvo
    Waa = Wfa @ Wa                 Wva = Wfv @ Wv
    b   = Wfa @ ba + Wfv @ bv + (Wf[:, :E] + Wf[:, E:]) @ bvo + bf

Weight composition is done on host in float64 (cheap: ~15 GFLOP), the big
GEMM (32768 x 4096 @ 4096 x 1024, 275 GFLOP) runs on 8 NeuronCores, batch
sharded (pure data parallel per the sharding hint).

Device layout per core:
    xt  [K=4096, BC=4096]  fp32  - per-core activations, feature-major
    wt  [K=4096, E=1024]   fp32  - composed weight, feature-major (replicated)
    bias[P=128,  E=1024]   fp32  - row-replicated bias
    out [BC=4096, E=1024]  fp32  - natural layout

Matmuls run as float32r (fp32 bits on the PE fast path: 1 cyc/row at
N>=256 vs 4 cyc/row for plain float32; ~1.3e-4 scale-relative absmax).
PSUM tile [128 batch, 512 outfeat]: stationary = xt subtile [128k, 128b]
(reused across the 2 outfeat halves to halve weight-load overhead),
moving = wt tile [128k, 512e]. Epilogue: DVE adds the row-replicated
bias PSUM->SBUF, ScalarE applies Relu, DMA out in natural layout.
"""

import os
import sys

import numpy as np

sys.path.insert(0, "/opt/trn_rl_repo")

import concourse.bacc as bacc
import concourse.mybir as mybir
import concourse.tile as tile
from concourse.bass_utils import run_bass_kernel_spmd

N_CORES = 8
B = 32768
BC = B // N_CORES  # 4096 batch rows per core
K = 4096           # 2048 audio + 2048 visual features
E = 1024
P = 128

KO = K // P        # 32 contraction tiles
NB = 512           # batch tile per iteration
NBT = BC // NB     # 8 batch tiles per core
B4 = NB // P       # 4 batch subtiles (PSUM partition dim)
M2 = E // NB       # 2 outfeat halves (PSUM free dim)

DT_NAME = os.environ.get("KMM_DTYPE", "bf16")

_NC_CACHE = {}
LAST_RESULTS = None  # stashed BassKernelResults for test.py introspection

# Note: walrus's --enable-ldw-opt=true was tested (dedupes the shared-lhsT
# LDWEIGHTS pairs, 2048 -> 1087) but measured SLOWER: the standalone-LW form
# loses the fused matmul's background weight-buffer pipelining (+12us PE).


def _build_nc(dt_name):
    mm_dt = {
        "f32": mybir.dt.float32,
        "f32r": mybir.dt.float32r,
        "bf16": mybir.dt.bfloat16,
    }[dt_name]
    f32 = mybir.dt.float32

    nc = bacc.Bacc("TRN2", debug=False, target_bir_lowering=False)
    xt = nc.dram_tensor("xt", [K, BC], mm_dt, kind="ExternalInput").ap()
    wt = nc.dram_tensor("wt", [K, E], mm_dt, kind="ExternalInput").ap()
    bias = nc.dram_tensor("bias", [P, E], f32, kind="ExternalInput").ap()
    out = nc.dram_tensor("out", [BC, E], f32, kind="ExternalOutput").ap()

    with tile.TileContext(nc) as tc:
        with (
            tc.tile_pool(name="wpool", bufs=1) as wpool,
            tc.tile_pool(name="xpool", bufs=12) as xpool,
            tc.tile_pool(name="opool", bufs=8) as opool,
            tc.tile_pool(name="pspool", bufs=8, space="PSUM") as pspool,
        ):
            # The DMA path is one FIFO queue fanned over 16 engines (~300-346
            # GB/s measured; bigger transfers amortize per-DMA overhead):
            # arrival order == emission order. Order the preamble
            # just-in-time for batch tile 0's k-sweep: first 8 xch (PE can
            # start), weight chunks 0-7, then alternating (xch, weight)
            # pairs so neither stream starves the other. Emitting the whole
            # 16.8 MB weight load first stalls PE ~60 us.
            wt_sb = wpool.tile([P, KO, E], mm_dt)
            wt_r = wt.rearrange("(ko ki) e -> ki ko e", ki=P)
            xch0 = {}
            for k in range(8):
                xch = xpool.tile([P, NB], mm_dt, tag="xch")
                nc.sync.dma_start(xch, xt[k * P : (k + 1) * P, 0:NB])
                xch0[k] = xch
                nc.sync.dma_start(wt_sb[:, k], wt_r[:, k])

            bias_sb = wpool.tile([P, E], f32)
            for k in range(8, KO):
                xch = xpool.tile([P, NB], mm_dt, tag="xch")
                nc.sync.dma_start(xch, xt[k * P : (k + 1) * P, 0:NB])
                xch0[k] = xch
                if k % 4 == 0:
                    # 2 MB weight chunks (4 ko's): fewer, larger transfers
                    # raise effective DMA bandwidth in the bandwidth-bound
                    # first k-sweep.
                    nc.sync.dma_start(wt_sb[:, k : k + 4], wt_r[:, k : k + 4])
                if k == 11:
                    # Bias early enough for the first iteration's drains
                    # (~85us) but off the first weight chunks' critical path
                    # (row-replicated, [128, E]).
                    nc.sync.dma_start(bias_sb, bias)

            for n in range(NBT):
                psums = [
                    pspool.tile([P, NB], f32, tag="ps", name=f"ps_{n}_{j}")
                    for j in range(B4 * M2)
                ]
                for k in range(KO):
                    if n == 0:
                        xch = xch0[k]
                    else:
                        xch = xpool.tile([P, NB], mm_dt, tag="xch")
                        nc.sync.dma_start(
                            xch, xt[k * P : (k + 1) * P, n * NB : (n + 1) * NB]
                        )
                    for b in range(B4):
                        for m in range(M2):
                            nc.tensor.matmul(
                                psums[b * M2 + m],
                                lhsT=xch[:, b * P : (b + 1) * P],
                                rhs=wt_sb[:, k, m * NB : (m + 1) * NB],
                                start=(k == 0),
                                stop=(k == KO - 1),
                            )
                for b in range(B4):
                    for m in range(M2):
                        ps = psums[b * M2 + m]
                        osb = opool.tile([P, NB], f32, tag="osb")
                        nc.vector.tensor_add(
                            out=osb,
                            in0=ps,
                            in1=bias_sb[:, m * NB : (m + 1) * NB],
                        )
                        nc.scalar.activation(
                            osb, osb, mybir.ActivationFunctionType.Relu
                        )
                        nc.sync.dma_start(
                            out[
                                n * NB + b * P : n * NB + (b + 1) * P,
                                m * NB : (m + 1) * NB,
                            ],
                            osb,
                        )

    nc.compile()
    return nc


def _get_nc(dt_name):
    if dt_name not in _NC_CACHE:
        _NC_CACHE[dt_name] = _build_nc(dt_name)
    return _NC_CACHE[dt_name]


def _compose_weights(Wa, ba, Wv, bv, Wi, bi, Wo, bo, Wf, bf):
    f6 = lambda x: np.asarray(x, dtype=np.float64)
    Wvo = f6(Wo) @ f6(Wi[2 * E :])
    bvo = f6(Wo) @ f6(bi[2 * E :]) + f6(bo)
    Wf1, Wf2 = f6(Wf[:, :E]), f6(Wf[:, E:])
    Wfv = Wf1 @ Wvo  # applied to visual_e for audio_att
    Wfa = Wf2 @ Wvo  # applied to audio_e for visual_att
    Waa = Wfa @ f6(Wa)  # [E, 2048] applied to audio
    Wva = Wfv @ f6(Wv)  # [E, 2048] applied to visual
    b = Wfa @ f6(ba) + Wfv @ f6(bv) + (Wf1 + Wf2) @ bvo + f6(bf)
    wt = np.ascontiguousarray(
        np.concatenate([Waa, Wva], axis=1).T, dtype=np.float32
    )  # [K, E]
    return wt, b.astype(np.float32)


def kernel(audio, visual, Wa, ba, Wv, bv, Wi, bi, Wo, bo, Wf, bf):
    global LAST_RESULTS
    wt, bias = _compose_weights(Wa, ba, Wv, bv, Wi, bi, Wo, bo, Wf, bf)
    bias_bc = np.ascontiguousarray(np.broadcast_to(bias, (P, E)), np.float32)
    np_mm = mybir.dt.np(
        {
            "f32": mybir.dt.float32,
            "f32r": mybir.dt.float32r,
            "bf16": mybir.dt.bfloat16,
        }[DT_NAME]
    )
    audio = np.asarray(audio, dtype=np.float32).astype(np_mm)
    visual = np.asarray(visual, dtype=np.float32).astype(np_mm)
    wt = wt.astype(np_mm)

    in_maps = []
    for c in range(N_CORES):
        rows = slice(c * BC, (c + 1) * BC)
        xt_c = np.empty((K, BC), np_mm)
        xt_c[: K // 2] = audio[rows].T
        xt_c[K // 2 :] = visual[rows].T
        in_maps.append({"xt": xt_c, "wt": wt, "bias": bias_bc})

    nc = _get_nc(DT_NAME)
    trace = os.environ.get("KMM_TRACE", "0") == "1"
    kwargs = {}
    if os.environ.get("KMM_TRACE_ALL", "0") == "1":
        kwargs["trace_cores"] = list(range(N_CORES))
    res = run_bass_kernel_spmd(
        nc, in_maps, core_ids=list(range(N_CORES)), trace=trace, **kwargs
    )
    LAST_RESULTS = res
    out = np.concatenate([r["out"] for r in res.results], axis=0)
    return np.ascontiguousarray(out, dtype=np.float32)

